# revision 51
# baseline (speedup 1.0000x reference)
"""Trainium2 Bass kernel for a 3-layer distributed GraphSAGE
(100000 nodes, 600000 edges, feats 128 -> 128 -> 128 -> 64, mean aggregation).

Strategy: 8-way contiguous node partition.  Layer 0 gathers raw x rows
from host-staged window tables (no collective needed); layers 1-2
compute z = h@W_neigh per shard, replicate z with four window
AllGathers, gather the z rows for the in-edges with batched dma_gather
calls, and segment-sum them into PSUM via 0/1 selection-matrix matmuls.

v4 structure:
 - S selection matrices are 0/1 fp8 (half the stream of fp16); the 1/deg
   mean scaling is applied per dst range on the DVE from a resident
   table.  The slab is streamed per band as int32 so the DMA splitter
   emits 4x bigger descriptors.
 - Layers 0-1 accumulate the output PSUM feature-major (out = W^T @ hT),
   fusing bias+ReLU into one ACT op that writes hT[l+1] directly.
 - Layer 2 only computes/stores the first 64 z/out columns.
 - Uneven gather windows [30,30,30,8 ranges]: the last window's
   AllGather -- the one on the layer-boundary critical path -- is tiny.
 - xT loads are chunked per band; z-row stores go through the ACT HWDGE
   ring to keep the Sync ring free for S-band streams.
"""
import os
import sys

sys.path.insert(0, "/opt/trn_rl_repo")

import numpy as np


import concourse.bass as bass
import concourse.mybir as mybir
import concourse.tile as tile
from concourse.masks import make_identity

F32 = mybir.dt.float32
F16 = mybir.dt.float16
F8 = mybir.dt.float8e4
I16 = mybir.dt.int16
I32 = mybir.dt.int32
NP_F8 = mybir.dt.np(F8)

WINDOW_RANGES = [30, 30, 30, 8]


def _roundup(a, m):
    return (a + m - 1) // m * m


# ---------------------------------------------------------------- host prep
def prepare(x, src, dst, n_cores=8, band_ranges=8):
    n_nodes, in_feats = x.shape
    src = np.asarray(src, np.int64)
    dst = np.asarray(dst, np.int64)
    assert n_nodes % n_cores == 0
    shard = n_nodes // n_cores
    shard_pad = _roundup(shard, 128)
    n_ranges = shard_pad // 128
    assert sum(WINDOW_RANGES) == n_ranges
    q_ranges = []
    pos = 0
    for k in WINDOW_RANGES:
        q_ranges.append(list(range(pos, pos + k)))
        pos += k
    nw = len(q_ranges)
    q_rows = [len(qq) * 128 for qq in q_ranges]          # rows/core/window
    q_row0 = [qq[0] * 128 for qq in q_ranges]
    tbl_q = [n_cores * rw for rw in q_rows]
    assert all(t <= 32768 for t in tbl_q)

    deg = np.bincount(dst, minlength=n_nodes).astype(np.float32)
    inv_deg = (1.0 / np.maximum(deg, 1.0)).astype(np.float32)

    s_core = src // shard
    s_loc = src % shard
    w_ends = np.array([q_row0[w] + q_rows[w] for w in range(nw)])
    win_of = np.searchsorted(w_ends, s_loc, side="right")
    q_rows_a = np.asarray(q_rows)[win_of]
    q_row0_a = np.asarray(q_row0)[win_of]
    idx_in_win = s_core * q_rows_a + (s_loc - q_row0_a)

    core_of = dst // shard
    d_loc = dst - core_of * shard
    rng_of = d_loc // 128

    counts = np.zeros((n_cores, n_ranges, nw), np.int64)
    np.add.at(counts, (core_of, rng_of, win_of), 1)
    asz = counts.max(axis=0)                  # common section sizes, unrounded

    bands = [list(range(b, min(b + band_ranges, n_ranges)))
             for b in range(0, n_ranges, band_ranges)]
    # Gather groups: one band per group (call granularity == band).
    ggroups = [[g] for g in range(len(bands))]
    group_of_band = {}
    for gi, gg in enumerate(ggroups):
        for b in gg:
            group_of_band[b] = gi
    calls = []
    slot_cursor = 0
    n_units = 0
    for gi, gg in enumerate(ggroups):
        granges = [r for b in gg for r in bands[b]]
        for w in range(nw):
            sec = int(sum(asz[r, w] for r in granges))
            if sec == 0:
                continue
            nslots = _roundup(sec, 128)
            secs, off = [], 0
            for r in granges:
                if asz[r, w]:
                    secs.append((int(r), off, off + int(asz[r, w])))
                    off += int(asz[r, w])
            units = []       # (local subtile, range, unit id)
            for (r, lo, hi) in secs:
                for t in range(lo // 128, (hi + 127) // 128):
                    units.append((t, r, n_units))
                    n_units += 1
            calls.append(dict(w=w, slot0=slot_cursor, nslots=nslots,
                              secs=secs, units=units, group=gi))
            slot_cursor += nslots
    total_slots = slot_cursor

    # per-range schedule: (call idx, unit id, local subtile), in w order
    per_range = [[] for _ in range(n_ranges)]
    for ci, call in enumerate(calls):
        for (t, r, u) in call["units"]:
            per_range[r].append((ci, u, t))

    group_unit = []
    for gi in range(len(ggroups)):
        cs = [c for c in calls if c["group"] == gi]
        us = [u for c in cs for (_, _, u) in c["units"]]
        group_unit.append((min(us), max(us) - min(us) + 1))

    per_core = []
    for c in range(n_cores):
        m = core_of == c
        e_idx = idx_in_win[m]
        e_dl = d_loc[m]
        e_w = win_of[m]
        key = (e_dl // 128) * nw + e_w
        order = np.argsort(key, kind="stable")
        e_idx, e_dl, e_w = (a[order] for a in (e_idx, e_dl, e_w))

        e_src = src[m][order]           # global src node per edge

        idx16 = np.zeros(total_slots, np.int16)
        gsrc = np.zeros(total_slots, np.int64)
        S = np.zeros((n_units, 128, 128), NP_F8)
        cnt = counts[c]
        pos = 0
        gstart = {}
        for r in range(n_ranges):
            for w in range(nw):
                gstart[(r, w)] = pos
                pos += int(cnt[r, w])
        assert pos == m.sum()
        one8 = NP_F8(1.0)
        for ci, call in enumerate(calls):
            w = call["w"]
            s0 = call["slot0"]
            u_of = {(t, r): u for (t, r, u) in call["units"]}
            for (r, lo, hi) in call["secs"]:
                k = int(cnt[r, w])
                if k == 0:
                    continue
                e0 = gstart[(r, w)]
                sl = slice(e0, e0 + k)
                slots = np.arange(lo, lo + k)
                idx16[s0 + lo:s0 + lo + k] = e_idx[sl].astype(np.int16)
                gsrc[s0 + lo:s0 + lo + k] = e_src[sl]
                t0 = lo // 128
                uids = np.array([u_of[(t, r)] for t in
                                 range(t0, (hi + 127) // 128)])
                u_arr = uids[slots // 128 - t0]
                S[u_arr, slots % 128, (e_dl[sl] - r * 128)] = one8

        idx_img = np.tile(idx16.reshape(-1, 16).T, (8, 1))
        s_img = np.ascontiguousarray(
            S.transpose(1, 0, 2).reshape(128, n_units * 128)
        ).view(np.int32)
        # host-pregathered layer-0 rows, in slot order, g-tile layout:
        # partition p column-block s holds x[src(slot s*128+p)]
        x16g = x.astype(np.float16)[gsrc]           # [total_slots, 128]
        xg_img = np.ascontiguousarray(
            x16g.reshape(total_slots // 128, 128, 128)
            .transpose(1, 0, 2).reshape(128, total_slots * 128 // 128)
        ).view(np.int32)

        invd = np.ones((128, n_ranges), np.float32)
        base = c * shard
        for r in range(n_ranges):
            lo = r * 128
            hi = min(lo + 128, shard)
            if hi > lo:
                invd[: hi - lo, r] = inv_deg[base + lo: base + hi]

        xt = np.zeros((in_feats, shard_pad), np.float16)
        xt[:, :shard] = x[c * shard:(c + 1) * shard].T.astype(np.float16)
        per_core.append(dict(xT=xt, idx_img=idx_img, s_img=s_img, invd=invd,
                             xg_img=xg_img))

    ones1 = np.ones((1, 128), np.float16)
    for pc in per_core:
        pc["ones1"] = ones1

    meta = dict(n_cores=n_cores, shard=shard, shard_pad=shard_pad,
                n_ranges=n_ranges, q_ranges=q_ranges, q_rows=q_rows,
                q_row0=q_row0, tbl_q=tbl_q, nw=nw, bands=bands, calls=calls,
                per_range=per_range, group_unit=group_unit, n_units=n_units,
                ggroups=ggroups, group_of_band=group_of_band,
                total_slots=total_slots, in_feats=in_feats)
    return meta, per_core


# ------------------------------------------------------------- kernel build
def build_kernel(nc, meta, n_classes):
    P = 128
    shard, shard_pad = meta["shard"], meta["shard_pad"]
    fin0 = meta["in_feats"]

    xT = nc.dram_tensor("xT", [fin0, shard_pad], F16, kind="ExternalInput").ap()
    xg_d = nc.dram_tensor("xg_img", [P, meta["total_slots"] // 2], I32,
                          kind="ExternalInput").ap()
    idx_d = nc.dram_tensor("idx_img", [P, meta["total_slots"] // 16], I16,
                           kind="ExternalInput").ap()
    s_d = nc.dram_tensor("s_img", [P, meta["n_units"] * 32], I32,
                         kind="ExternalInput").ap()
    invd_d = nc.dram_tensor("invd", [P, meta["n_ranges"]], F32,
                            kind="ExternalInput").ap()
    ones_d = nc.dram_tensor("ones1", [1, P], F16, kind="ExternalInput").ap()
    w_specs = [("W_self0", [P, P], F16), ("W_neigh0", [P, P], F16),
               ("W_self1", [P, P], F16), ("W_neigh1", [P, P], F16),
               ("W_self2", [P, n_classes], F16),
               ("W_neigh2", [P, n_classes], F16),
               ("b0", [P, 1], F32), ("b1", [P, 1], F32),
               ("b2", [1, n_classes], F16)]
    w_d = {name: (nc.dram_tensor(name, shape, dt, kind="ExternalInput").ap(),
                  shape, dt)
           for name, shape, dt in w_specs}
    out_d = nc.dram_tensor("out", [shard, n_classes], F32,
                           kind="ExternalOutput").ap()

    with tile.TileContext(nc) as tc:
        import contextlib
        with contextlib.ExitStack() as ctx:
            _body(ctx, tc, meta, n_classes, xT, xg_d, idx_d, s_d, invd_d,
                  ones_d, w_d, out_d)
    return nc


def _body(ctx, tc, meta, n_classes, xT, xg_d, idx_d, s_d, invd_d, ones_d,
          w_d, out_d):
    P = 128
    NC = n_classes
    nc = tc.nc
    shard, shard_pad = meta["shard"], meta["shard_pad"]
    n_ranges = meta["n_ranges"]
    q_ranges, q_rows = meta["q_ranges"], meta["q_rows"]
    tbl_q, nw = meta["tbl_q"], meta["nw"]
    q_of_range = {}
    for q, qq in enumerate(q_ranges):
        for r in qq:
            q_of_range[r] = q
    q_last_range = [qq[-1] for qq in q_ranges]
    calls, per_range = meta["calls"], meta["per_range"]
    group_unit = meta["group_unit"]
    group_of_band = meta["group_of_band"]
    total_slots = meta["total_slots"]
    rg = [list(range(meta["n_cores"]))]
    max_call_sub = max(c["nslots"] for c in calls) // 128
    max_group_unit = max(n for _, n in group_unit)

    pers = ctx.enter_context(tc.tile_pool(name="pers", bufs=1))
    dram = ctx.enter_context(tc.tile_pool(name="dram", bufs=1, space="DRAM"))
    gpool = ctx.enter_context(tc.tile_pool(name="gp", bufs=28))
    sld = ctx.enter_context(tc.tile_pool(name="sld", bufs=2))
    hpool = ctx.enter_context(tc.tile_pool(name="hp", bufs=2))
    rpool = ctx.enter_context(tc.tile_pool(name="rp", bufs=4))
    apool = ctx.enter_context(tc.tile_pool(name="ap", bufs=2, space="PSUM"))
    bpool = ctx.enter_context(tc.tile_pool(name="bp", bufs=2, space="PSUM"))
    tpool = ctx.enter_context(tc.tile_pool(name="tp", bufs=2, space="PSUM"))
    zpool = ctx.enter_context(tc.tile_pool(name="zp", bufs=2, space="PSUM"))

    # warmup collective first: its input DMA must not queue behind the
    # bulk prelude loads, so the CC core is free before the first z AG.
    wu_in = dram.tile([P, 1], F32, name="wu_in")
    wu_out = dram.tile([P * meta["n_cores"], 1], F32, addr_space="Shared",
                       name="wu_out")
    wu_sb = pers.tile([P, 1], F32, name="wu_sb")
    nc.vector.memset(wu_sb[:], 0.0)
    nc.sync.dma_start(out=wu_in[:], in_=wu_sb[:])
    nc.gpsimd.collective_compute("AllGather", mybir.AluOpType.bypass,
                                 replica_groups=rg, ins=[wu_in[:]],
                                 outs=[wu_out[:]])

    idx_sb = pers.tile([P, total_slots // 16], I16, name="idx_sb")
    nc.sync.dma_start(out=idx_sb[:], in_=idx_d[:])
    invd_sb = pers.tile([P, n_ranges], F32, name="invd_sb")
    nc.sync.dma_start(out=invd_sb[:], in_=invd_d[:])
    ones_sb = pers.tile([1, P], F16, name="ones_sb")
    nc.sync.dma_start(out=ones_sb[:], in_=ones_d[:])
    ident = pers.tile([P, P], F16, name="ident")
    make_identity(nc, ident[:])
    w_sb = {}
    for name, (ap_, shape, dt) in w_d.items():
        t = pers.tile(shape, dt, name=name)
        nc.sync.dma_start(out=t[:], in_=ap_[:])
        w_sb[name] = t
    Ws = [w_sb["W_self0"], w_sb["W_self1"], w_sb["W_self2"]]
    Wn = [w_sb["W_neigh0"], w_sb["W_neigh1"], w_sb["W_neigh2"]]

    hT = [None] * 3
    hT[0] = hpool.tile([P, shard_pad], F16, name="hT0", tag="hT")
    for l in range(1, 3):
        hT[l] = hpool.tile([P, shard_pad], F16, name=f"hT{l}", tag="hT")

    # z tables (gather rows must stride 256B, so both are 128-wide fp16;
    # layer 2 only touches the first 64 columns).
    zw = [None, P, NC]                 # useful width per layer
    zbq = [None] + [[dram.tile([q_rows[q], P], F16, name=f"zb{l}_{q}")
                     for q in range(nw)] for l in range(1, 3)]
    zfq_t = [None] + [[dram.tile([tbl_q[q], P], F16, addr_space="Shared",
                                 name=f"zf{l}_{q}")
                       for q in range(nw)] for l in range(1, 3)]

    pending_ag = []     # deferred AllGather triggers: [z_layer, q, band]
    qn = [0]
    gtiles = {}         # (layer, call idx) -> gather tile
    call_of = {(c["group"], c["w"]): ci for ci, c in enumerate(calls)}

    def issue_call(gl, ci):
        """Issue the slot-data fetch for (layer gl, call ci) once."""
        if (gl, ci) in gtiles:
            return
        c = calls[ci]
        nsub_c = c["nslots"] // 128
        if gl == 0:
            # layer 0: stream host-pregathered rows (no SWDGE)
            g = gpool.tile([P, nsub_c * 64], I32, name="g", tag="g",
                           padded_shape=[P, max_call_sub * 64])
            nc.sync.dma_start(out=g[:],
                              in_=xg_d[:, c["slot0"] // 2:
                                       (c["slot0"] + c["nslots"]) // 2])
        else:
            g = gpool.tile([P, nsub_c, P], F16, name="g", tag="g",
                           padded_shape=[P, max_call_sub, P])
            nc.gpsimd.dma_gather(
                out_ap=g[:], in_ap=zfq_t[gl][c["w"]][:, :],
                idxs_ap=idx_sb[:, c["slot0"] // 16:
                               (c["slot0"] + c["nslots"]) // 16],
                num_idxs=c["nslots"], num_idxs_reg=c["nslots"],
                elem_size=P, single_packet=False,
                queue_num=qn[0] % nc.num_swdge_queues)
            qn[0] += 1
        gtiles[(gl, ci)] = g

    def ag_trigger(zl, q):
        nc.gpsimd.collective_compute(
            "AllGather", mybir.AluOpType.bypass, replica_groups=rg,
            ins=[zbq[zl][q][:]], outs=[zfq_t[zl][q][:]])
        # prefetch layer zl's window-q gathers for its first bands: they
        # generate+drain during the producing layer's tail, while the Q7
        # is otherwise idle, instead of piling up at the layer entry.
        if q < nw - 1:
            for b in range(min(4, len(meta["bands"]))):
                issue_call(zl, call_of[(b, q)])

    def emit_z(l, r, bi):
        """Produce z_l rows for range r of h_l.  z1's window AllGathers
        fire immediately (the z1 chain is the l0-phase floor); z2's are
        deferred two bands so the trigger's store-completion wait never
        blocks the Pool queue behind pending gathers."""
        width = zw[l]
        pz = zpool.tile([P, 512], F32, name="pz", tag="pz")
        nc.tensor.matmul(out=pz[:, :width], lhsT=hT[l][:, r * P:(r + 1) * P],
                         rhs=Wn[l][:, :width], start=True, stop=True)
        zrow = rpool.tile([P, width], F16, name="zrow", tag="zrow",
                          padded_shape=[P, P])
        nc.scalar.activation(out=zrow[:], in_=pz[:, :width],
                             func=mybir.ActivationFunctionType.Copy)
        q = q_of_range[r]
        r2 = r - q_ranges[q][0]
        nc.scalar.dma_start(out=zbq[l][q][r2 * P:(r2 + 1) * P, :width],
                            in_=zrow[:])
        if r == q_last_range[q]:
            if l == 1 or q == nw - 1:
                ag_trigger(l, q)
            else:
                pending_ag.append([l, q, bi])

    for l in range(3):
        last = l == 2
        aw = NC if last else P          # aggregation width
        for bi, band in enumerate(meta["bands"]):
            # fire deferred AllGather triggers whose stores are long done
            for ent in list(pending_ag):
                if ent[0] <= l or ent[2] <= bi - 2:
                    ag_trigger(ent[0], ent[1])
                    pending_ag.remove(ent)
            gi = group_of_band[bi]
            gu0, gun = group_unit[gi]
            if bi == meta["ggroups"][gi][0]:
                # group start: stream the S slab + issue the group's gathers
                sband = sld.tile([P, gun * 32], I32, name="sband",
                                 tag="sband",
                                 padded_shape=[P, max_group_unit * 32])
                nc.sync.dma_start(out=sband[:],
                                  in_=s_d[:, gu0 * 32:(gu0 + gun) * 32])
                sb8 = sband[:].bitcast(F8)
                for ci, c in enumerate(calls):
                    if c["group"] == gi:
                        issue_call(l, ci)
            if l == 0:
                # just-in-time chunk of the transposed input features
                lo = band[0] * P
                hi = min((band[-1] + 1) * P, shard_pad)
                nc.sync.dma_start(out=hT[0][:, lo:hi], in_=xT[:, lo:hi])

            abanks = [apool.tile([P, 512], F32, name="agg", tag="agg")
                      for _ in range((len(band) + 3) // 4)]
            bbanks = [bpool.tile([P, 512], F32, name="outp", tag="outp")
                      for _ in range((len(band) + 3) // 4)]
            for j, r in enumerate(band):
                aslice = abanks[j // 4][:, (j % 4) * P:(j % 4) * P + aw]
                units = per_range[r]
                for k, (ci, u, t) in enumerate(units):
                    g = gtiles[(l, ci)]
                    su = u - gu0
                    rhs = (g[:, t * 64:(t + 1) * 64].bitcast(F16)
                           if l == 0 else g[:, t, :aw])
                    nc.tensor.matmul(
                        out=aslice, lhsT=sb8[:, su * P:(su + 1) * P],
                        rhs=rhs, start=(k == 0),
                        stop=(k == len(units) - 1))
                # mean scaling: sa = agg * (1/deg), per-partition scalar
                sa = rpool.tile([P, aw], F16, name="sa", tag="sa",
                                padded_shape=[P, P])
                nc.vector.tensor_scalar(
                    out=sa[:], in0=aslice, scalar1=invd_sb[:, r:r + 1],
                    scalar2=None, op0=mybir.AluOpType.mult)
                if l == 0:
                    # transpose scaled agg, then W_neigh0^T @ aggT
                    paT = tpool.tile([P, 512], F16, name="paT", tag="pt")
                    nc.tensor.transpose(out=paT[:P, :P], in_=sa[:],
                                        identity=ident[:])
                    saT = rpool.tile([P, P], F16, name="saT", tag="saT")
                    nc.vector.tensor_copy(out=saT[:], in_=paT[:P, :P])
                    bslice = bbanks[j // 4][:, (j % 4) * P:(j % 4 + 1) * P]
                    nc.tensor.matmul(out=bslice, lhsT=Wn[0][:], rhs=saT[:],
                                     start=True, stop=False)
                    nc.tensor.matmul(out=bslice, lhsT=Ws[0][:],
                                     rhs=hT[0][:, r * P:(r + 1) * P],
                                     start=False, stop=True)
                    nc.scalar.activation(
                        out=hT[1][:, r * P:(r + 1) * P], in_=bslice,
                        func=mybir.ActivationFunctionType.Relu,
                        bias=w_sb["b0"][:])
                    emit_z(1, r, bi)
                elif l == 1:
                    # feature-major: B = sa^T + Ws1^T @ hT1
                    bslice = bbanks[j // 4][:, (j % 4) * P:(j % 4 + 1) * P]
                    nc.tensor.matmul(out=bslice, lhsT=sa[:], rhs=ident[:],
                                     start=True, stop=False)
                    nc.tensor.matmul(out=bslice, lhsT=Ws[1][:],
                                     rhs=hT[1][:, r * P:(r + 1) * P],
                                     start=False, stop=True)
                    nc.scalar.activation(
                        out=hT[2][:, r * P:(r + 1) * P], in_=bslice,
                        func=mybir.ActivationFunctionType.Relu,
                        bias=w_sb["b1"][:])
                    emit_z(2, r, bi)
                else:
                    # node-major final layer: B = sa + ones@b2 + hT2^T@Ws2
                    bslice = bbanks[j // 4][:, (j % 4) * P:(j % 4) * P + NC]
                    nc.tensor.matmul(out=bslice, lhsT=ident[:], rhs=sa[:],
                                     start=True, stop=False)
                    nc.tensor.matmul(out=bslice, lhsT=ones_sb[:],
                                     rhs=w_sb["b2"][:], start=False,
                                     stop=False)
                    nc.tensor.matmul(out=bslice,
                                     lhsT=hT[2][:, r * P:(r + 1) * P],
                                     rhs=Ws[2][:], start=False, stop=True)
                    rowt = rpool.tile([P, NC], F32, name="rowt", tag="rowt")
                    nc.scalar.activation(
                        out=rowt[:], in_=bslice,
                        func=mybir.ActivationFunctionType.Copy)
                    r0 = r * P
                    nrows = min(shard - r0, P)
                    if nrows > 0:
                        nc.sync.dma_start(out=out_d[r0:r0 + nrows, :],
                                          in_=rowt[:nrows, :])


# ----------------------------------------------------------------- runner
N_CORES = 8
N_NODES = 100000
N_EDGES = 600000
IN_FEATS = 128
N_HIDDEN = 128
N_CLASSES = 64

_TRACE_RESULT = {}


def kernel(x, src, dst, W_self0, W_neigh0, b0, W_self1, W_neigh1, b1,
           W_self2, W_neigh2, b2):
    import concourse.bacc as bacc
    from concourse import bass_utils

    x = np.asarray(x, np.float32)
    src = np.asarray(src, np.int64)
    dst = np.asarray(dst, np.int64)
    assert x.shape == (N_NODES, IN_FEATS)
    assert src.shape == (N_EDGES,) and dst.shape == (N_EDGES,)

    meta, per_core = prepare(x, src, dst, n_cores=N_CORES)

    wpack = {
        "W_self0": np.asarray(W_self0, np.float16),
        "W_neigh0": np.asarray(W_neigh0, np.float16),
        "W_self1": np.asarray(W_self1, np.float16),
        "W_neigh1": np.asarray(W_neigh1, np.float16),
        "W_self2": np.asarray(W_self2, np.float16),
        "W_neigh2": np.asarray(W_neigh2, np.float16),
        "b0": np.asarray(b0, np.float32).reshape(-1, 1),
        "b1": np.asarray(b1, np.float32).reshape(-1, 1),
        "b2": np.asarray(b2, np.float16).reshape(1, -1),
    }

    nc = bacc.Bacc("TRN2", target_bir_lowering=False, debug=False,
                   num_devices=N_CORES, num_swdge_queues=4)
    build_kernel(nc, meta, N_CLASSES)
    nc.compile()

    in_maps = []
    for c in range(N_CORES):
        pc = per_core[c]
        im = dict(xT=pc["xT"], idx_img=pc["idx_img"], s_img=pc["s_img"],
                  invd=pc["invd"], ones1=pc["ones1"], xg_img=pc["xg_img"])
        im.update(wpack)
        in_maps.append(im)

    trace = os.environ.get("SAGE_TRACE") == "1"
    res = bass_utils.run_bass_kernel_spmd(
        nc, in_maps, core_ids=list(range(N_CORES)), trace=trace)
    if trace:
        _TRACE_RESULT["exec_time_ns"] = res.exec_time_ns

    out = np.concatenate([res.results[c]["out"] for c in range(N_CORES)], 0)
    return np.ascontiguousarray(out[:N_NODES], np.float32)


# revision 53
# speedup vs baseline: 1.1598x; 1.1598x over previous
"""Trainium2 Bass kernel for a 3-layer distributed GraphSAGE
(100000 nodes, 600000 edges, feats 128 -> 128 -> 128 -> 64, mean aggregation).

Strategy: 8-way contiguous node partition.  Layer 0 gathers raw x rows
from host-staged window tables (no collective needed); layers 1-2
compute z = h@W_neigh per shard, replicate z with four window
AllGathers, gather the z rows for the in-edges with batched dma_gather
calls, and segment-sum them into PSUM via 0/1 selection-matrix matmuls.

v4 structure:
 - S selection matrices are 0/1 fp8 (half the stream of fp16); the 1/deg
   mean scaling is applied per dst range on the DVE from a resident
   table.  The slab is streamed per band as int32 so the DMA splitter
   emits 4x bigger descriptors.
 - Layers 0-1 accumulate the output PSUM feature-major (out = W^T @ hT),
   fusing bias+ReLU into one ACT op that writes hT[l+1] directly.
 - Layer 2 only computes/stores the first 64 z/out columns.
 - Uneven gather windows [30,30,30,8 ranges]: the last window's
   AllGather -- the one on the layer-boundary critical path -- is tiny.
 - xT loads are chunked per band; z-row stores go through the ACT HWDGE
   ring to keep the Sync ring free for S-band streams.
"""
import os
import sys

sys.path.insert(0, "/opt/trn_rl_repo")

import numpy as np


import concourse.bass as bass
import concourse.mybir as mybir
import concourse.tile as tile
from concourse.masks import make_identity

F32 = mybir.dt.float32
F16 = mybir.dt.float16
F8 = mybir.dt.float8e4
I16 = mybir.dt.int16
I32 = mybir.dt.int32
NP_F8 = mybir.dt.np(F8)

WINDOW_RANGES = [30, 30, 30, 8]


def _roundup(a, m):
    return (a + m - 1) // m * m


# ---------------------------------------------------------------- host prep
def prepare(x, src, dst, n_cores=8, band_ranges=8):
    n_nodes, in_feats = x.shape
    src = np.asarray(src, np.int64)
    dst = np.asarray(dst, np.int64)
    assert n_nodes % n_cores == 0
    shard = n_nodes // n_cores
    shard_pad = _roundup(shard, 128)
    n_ranges = shard_pad // 128
    assert sum(WINDOW_RANGES) == n_ranges
    q_ranges = []
    pos = 0
    for k in WINDOW_RANGES:
        q_ranges.append(list(range(pos, pos + k)))
        pos += k
    nw = len(q_ranges)
    q_rows = [len(qq) * 128 for qq in q_ranges]          # rows/core/window
    q_row0 = [qq[0] * 128 for qq in q_ranges]
    tbl_q = [n_cores * rw for rw in q_rows]
    assert all(t <= 32768 for t in tbl_q)

    deg = np.bincount(dst, minlength=n_nodes).astype(np.float32)
    inv_deg = (1.0 / np.maximum(deg, 1.0)).astype(np.float32)

    s_core = src // shard
    s_loc = src % shard
    w_ends = np.array([q_row0[w] + q_rows[w] for w in range(nw)])
    win_of = np.searchsorted(w_ends, s_loc, side="right")
    q_rows_a = np.asarray(q_rows)[win_of]
    q_row0_a = np.asarray(q_row0)[win_of]
    idx_in_win = s_core * q_rows_a + (s_loc - q_row0_a)

    core_of = dst // shard
    d_loc = dst - core_of * shard
    rng_of = d_loc // 128

    counts = np.zeros((n_cores, n_ranges, nw), np.int64)
    np.add.at(counts, (core_of, rng_of, win_of), 1)
    asz = counts.max(axis=0)                  # common section sizes, unrounded

    bands = [list(range(b, min(b + band_ranges, n_ranges)))
             for b in range(0, n_ranges, band_ranges)]
    # Gather groups: one band per group (call granularity == band).
    ggroups = [[g] for g in range(len(bands))]
    group_of_band = {}
    for gi, gg in enumerate(ggroups):
        for b in gg:
            group_of_band[b] = gi
    calls = []
    slot_cursor = 0
    n_units = 0
    for gi, gg in enumerate(ggroups):
        granges = [r for b in gg for r in bands[b]]
        for w in range(nw):
            sec = int(sum(asz[r, w] for r in granges))
            if sec == 0:
                continue
            nslots = _roundup(sec, 128)
            secs, off = [], 0
            for r in granges:
                if asz[r, w]:
                    secs.append((int(r), off, off + int(asz[r, w])))
                    off += int(asz[r, w])
            units = []       # (local subtile, range, unit id)
            for (r, lo, hi) in secs:
                for t in range(lo // 128, (hi + 127) // 128):
                    units.append((t, r, n_units))
                    n_units += 1
            calls.append(dict(w=w, slot0=slot_cursor, nslots=nslots,
                              secs=secs, units=units, group=gi))
            slot_cursor += nslots
    total_slots = slot_cursor

    # per-range schedule: (call idx, unit id, local subtile), in w order
    per_range = [[] for _ in range(n_ranges)]
    for ci, call in enumerate(calls):
        for (t, r, u) in call["units"]:
            per_range[r].append((ci, u, t))

    group_unit = []
    for gi in range(len(ggroups)):
        cs = [c for c in calls if c["group"] == gi]
        us = [u for c in cs for (_, _, u) in c["units"]]
        group_unit.append((min(us), max(us) - min(us) + 1))

    per_core = []
    for c in range(n_cores):
        m = core_of == c
        e_idx = idx_in_win[m]
        e_dl = d_loc[m]
        e_w = win_of[m]
        key = (e_dl // 128) * nw + e_w
        order = np.argsort(key, kind="stable")
        e_idx, e_dl, e_w = (a[order] for a in (e_idx, e_dl, e_w))

        e_src = src[m][order]           # global src node per edge

        idx16 = np.zeros(total_slots, np.int16)
        gsrc = np.zeros(total_slots, np.int64)
        S = np.zeros((n_units, 128, 128), NP_F8)
        cnt = counts[c]
        pos = 0
        gstart = {}
        for r in range(n_ranges):
            for w in range(nw):
                gstart[(r, w)] = pos
                pos += int(cnt[r, w])
        assert pos == m.sum()
        one8 = NP_F8(1.0)
        for ci, call in enumerate(calls):
            w = call["w"]
            s0 = call["slot0"]
            u_of = {(t, r): u for (t, r, u) in call["units"]}
            for (r, lo, hi) in call["secs"]:
                k = int(cnt[r, w])
                if k == 0:
                    continue
                e0 = gstart[(r, w)]
                sl = slice(e0, e0 + k)
                slots = np.arange(lo, lo + k)
                idx16[s0 + lo:s0 + lo + k] = e_idx[sl].astype(np.int16)
                gsrc[s0 + lo:s0 + lo + k] = e_src[sl]
                t0 = lo // 128
                uids = np.array([u_of[(t, r)] for t in
                                 range(t0, (hi + 127) // 128)])
                u_arr = uids[slots // 128 - t0]
                S[u_arr, slots % 128, (e_dl[sl] - r * 128)] = one8

        idx_img = np.tile(idx16.reshape(-1, 16).T, (8, 1))
        s_img = np.ascontiguousarray(
            S.transpose(1, 0, 2).reshape(128, n_units * 128)
        ).view(np.int32)
        # host-pregathered layer-0 rows, in slot order, g-tile layout:
        # partition p column-block s holds x[src(slot s*128+p)]
        x16g = x.astype(np.float16)[gsrc]           # [total_slots, 128]
        xg_img = np.ascontiguousarray(
            x16g.reshape(total_slots // 128, 128, 128)
            .transpose(1, 0, 2).reshape(128, total_slots * 128 // 128)
        ).view(np.int32)

        invd = np.ones((128, n_ranges), np.float32)
        base = c * shard
        for r in range(n_ranges):
            lo = r * 128
            hi = min(lo + 128, shard)
            if hi > lo:
                invd[: hi - lo, r] = inv_deg[base + lo: base + hi]

        xt = np.zeros((in_feats, shard_pad), np.float16)
        xt[:, :shard] = x[c * shard:(c + 1) * shard].T.astype(np.float16)
        per_core.append(dict(xT=xt, idx_img=idx_img, s_img=s_img, invd=invd,
                             xg_img=xg_img))

    ones1 = np.ones((1, 128), np.float16)
    for pc in per_core:
        pc["ones1"] = ones1

    meta = dict(n_cores=n_cores, shard=shard, shard_pad=shard_pad,
                n_ranges=n_ranges, q_ranges=q_ranges, q_rows=q_rows,
                q_row0=q_row0, tbl_q=tbl_q, nw=nw, bands=bands, calls=calls,
                per_range=per_range, group_unit=group_unit, n_units=n_units,
                ggroups=ggroups, group_of_band=group_of_band,
                total_slots=total_slots, in_feats=in_feats)
    return meta, per_core


# ------------------------------------------------------------- kernel build
def build_kernel(nc, meta, n_classes):
    P = 128
    shard, shard_pad = meta["shard"], meta["shard_pad"]
    fin0 = meta["in_feats"]

    xT = nc.dram_tensor("xT", [fin0, shard_pad], F16, kind="ExternalInput").ap()
    xg_d = nc.dram_tensor("xg_img", [P, meta["total_slots"] // 2], I32,
                          kind="ExternalInput").ap()
    idx_d = nc.dram_tensor("idx_img", [P, meta["total_slots"] // 16], I16,
                           kind="ExternalInput").ap()
    s_d = nc.dram_tensor("s_img", [P, meta["n_units"] * 32], I32,
                         kind="ExternalInput").ap()
    invd_d = nc.dram_tensor("invd", [P, meta["n_ranges"]], F32,
                            kind="ExternalInput").ap()
    ones_d = nc.dram_tensor("ones1", [1, P], F16, kind="ExternalInput").ap()
    w_specs = [("W_self0", [P, P], F16), ("W_neigh0", [P, P], F16),
               ("W_self1", [P, P], F16), ("W_neigh1", [P, P], F16),
               ("W_self2", [P, n_classes], F16),
               ("W_neigh2", [P, n_classes], F16),
               ("b0", [P, 1], F32), ("b1", [P, 1], F32),
               ("b2", [1, n_classes], F16)]
    w_d = {name: (nc.dram_tensor(name, shape, dt, kind="ExternalInput").ap(),
                  shape, dt)
           for name, shape, dt in w_specs}
    out_d = nc.dram_tensor("out", [shard, n_classes], F32,
                           kind="ExternalOutput").ap()

    with tile.TileContext(nc) as tc:
        import contextlib
        with contextlib.ExitStack() as ctx:
            _body(ctx, tc, meta, n_classes, xT, xg_d, idx_d, s_d, invd_d,
                  ones_d, w_d, out_d)
    return nc


def _body(ctx, tc, meta, n_classes, xT, xg_d, idx_d, s_d, invd_d, ones_d,
          w_d, out_d):
    P = 128
    NC = n_classes
    nc = tc.nc
    shard, shard_pad = meta["shard"], meta["shard_pad"]
    n_ranges = meta["n_ranges"]
    q_ranges, q_rows = meta["q_ranges"], meta["q_rows"]
    tbl_q, nw = meta["tbl_q"], meta["nw"]
    q_of_range = {}
    for q, qq in enumerate(q_ranges):
        for r in qq:
            q_of_range[r] = q
    q_last_range = [qq[-1] for qq in q_ranges]
    calls, per_range = meta["calls"], meta["per_range"]
    group_unit = meta["group_unit"]
    group_of_band = meta["group_of_band"]
    total_slots = meta["total_slots"]
    rg = [list(range(meta["n_cores"]))]
    max_call_sub = max(c["nslots"] for c in calls) // 128
    max_group_unit = max(n for _, n in group_unit)

    pers = ctx.enter_context(tc.tile_pool(name="pers", bufs=1))
    dram = ctx.enter_context(tc.tile_pool(name="dram", bufs=1, space="DRAM"))
    gpool = ctx.enter_context(tc.tile_pool(name="gp", bufs=26))
    sld = ctx.enter_context(tc.tile_pool(name="sld", bufs=2))
    hpool = ctx.enter_context(tc.tile_pool(name="hp", bufs=2))
    rpool = ctx.enter_context(tc.tile_pool(name="rp", bufs=4))
    apool = ctx.enter_context(tc.tile_pool(name="ap", bufs=2, space="PSUM"))
    bpool = ctx.enter_context(tc.tile_pool(name="bp", bufs=2, space="PSUM"))
    tpool = ctx.enter_context(tc.tile_pool(name="tp", bufs=2, space="PSUM"))
    zpool = ctx.enter_context(tc.tile_pool(name="zp", bufs=2, space="PSUM"))

    # warmup collective first: its input DMA must not queue behind the
    # bulk prelude loads, so the CC core is free before the first z AG.
    wu_in = dram.tile([P, 1], F32, name="wu_in")
    wu_out = dram.tile([P * meta["n_cores"], 1], F32, addr_space="Shared",
                       name="wu_out")
    wu_sb = pers.tile([P, 1], F32, name="wu_sb")
    nc.vector.memset(wu_sb[:], 0.0)
    nc.sync.dma_start(out=wu_in[:], in_=wu_sb[:])
    nc.gpsimd.collective_compute("AllGather", mybir.AluOpType.bypass,
                                 replica_groups=rg, ins=[wu_in[:]],
                                 outs=[wu_out[:]])

    idx_sb = pers.tile([P, total_slots // 16], I16, name="idx_sb")
    nc.sync.dma_start(out=idx_sb[:], in_=idx_d[:])
    invd_sb = pers.tile([P, n_ranges], F32, name="invd_sb")
    nc.sync.dma_start(out=invd_sb[:], in_=invd_d[:])
    ones_sb = pers.tile([1, P], F16, name="ones_sb")
    nc.sync.dma_start(out=ones_sb[:], in_=ones_d[:])
    ident = pers.tile([P, P], F16, name="ident")
    make_identity(nc, ident[:])
    w_sb = {}
    for name, (ap_, shape, dt) in w_d.items():
        t = pers.tile(shape, dt, name=name)
        nc.sync.dma_start(out=t[:], in_=ap_[:])
        w_sb[name] = t
    Ws = [w_sb["W_self0"], w_sb["W_self1"], w_sb["W_self2"]]
    Wn = [w_sb["W_neigh0"], w_sb["W_neigh1"], w_sb["W_neigh2"]]

    hT = [None] * 3
    hT[0] = hpool.tile([P, shard_pad], F16, name="hT0", tag="hT")
    for l in range(1, 3):
        hT[l] = hpool.tile([P, shard_pad], F16, name=f"hT{l}", tag="hT")

    # z tables (gather rows must stride 256B, so both are 128-wide fp16;
    # layer 2 only touches the first 64 columns).
    zw = [None, P, NC]                 # useful width per layer
    zbq = [None] + [[dram.tile([q_rows[q], P], F16, name=f"zb{l}_{q}")
                     for q in range(nw)] for l in range(1, 3)]
    zfq_t = [None] + [[dram.tile([tbl_q[q], P], F16, addr_space="Shared",
                                 name=f"zf{l}_{q}")
                       for q in range(nw)] for l in range(1, 3)]

    pending_ag = []     # deferred AllGather triggers: [z_layer, q, band]
    qn = [0]
    gtiles = {}         # (layer, call idx) -> gather tile
    call_of = {(c["group"], c["w"]): ci for ci, c in enumerate(calls)}

    def issue_call(gl, ci):
        """Issue the slot-data fetch for (layer gl, call ci) once."""
        if (gl, ci) in gtiles:
            return
        c = calls[ci]
        nsub_c = c["nslots"] // 128
        if gl == 0:
            # layer 0: stream host-pregathered rows (no SWDGE)
            g = gpool.tile([P, nsub_c * 64], I32, name="g", tag="g",
                           padded_shape=[P, max_call_sub * 64])
            nc.sync.dma_start(out=g[:],
                              in_=xg_d[:, c["slot0"] // 2:
                                       (c["slot0"] + c["nslots"]) // 2])
        else:
            g = gpool.tile([P, nsub_c, P], F16, name="g", tag="g",
                           padded_shape=[P, max_call_sub, P])
            nc.gpsimd.dma_gather(
                out_ap=g[:], in_ap=zfq_t[gl][c["w"]][:, :],
                idxs_ap=idx_sb[:, c["slot0"] // 16:
                               (c["slot0"] + c["nslots"]) // 16],
                num_idxs=c["nslots"], num_idxs_reg=c["nslots"],
                elem_size=P, single_packet=False,
                queue_num=qn[0] % nc.num_swdge_queues)
            qn[0] += 1
        gtiles[(gl, ci)] = g

    def ag_trigger(zl, q):
        nc.gpsimd.collective_compute(
            "AllGather", mybir.AluOpType.bypass, replica_groups=rg,
            ins=[zbq[zl][q][:]], outs=[zfq_t[zl][q][:]])

    def emit_z(l, r, bi):
        """Produce z_l rows for range r of h_l.  z1's window AllGathers
        fire immediately (the z1 chain is the l0-phase floor); z2's are
        deferred two bands so the trigger's store-completion wait never
        blocks the Pool queue behind pending gathers."""
        width = zw[l]
        pz = zpool.tile([P, 512], F32, name="pz", tag="pz")
        nc.tensor.matmul(out=pz[:, :width], lhsT=hT[l][:, r * P:(r + 1) * P],
                         rhs=Wn[l][:, :width], start=True, stop=True)
        zrow = rpool.tile([P, width], F16, name="zrow", tag="zrow",
                          padded_shape=[P, P])
        nc.scalar.activation(out=zrow[:], in_=pz[:, :width],
                             func=mybir.ActivationFunctionType.Copy)
        q = q_of_range[r]
        r2 = r - q_ranges[q][0]
        nc.scalar.dma_start(out=zbq[l][q][r2 * P:(r2 + 1) * P, :width],
                            in_=zrow[:])
        if r == q_last_range[q]:
            if l == 1 or q == nw - 1:
                ag_trigger(l, q)
            else:
                pending_ag.append([l, q, bi])

    for l in range(3):
        last = l == 2
        aw = NC if last else P          # aggregation width
        for bi, band in enumerate(meta["bands"]):
            # fire deferred AllGather triggers whose stores are long done
            for ent in list(pending_ag):
                if ent[0] <= l or ent[2] <= bi - 2:
                    ag_trigger(ent[0], ent[1])
                    pending_ag.remove(ent)
            gi = group_of_band[bi]
            gu0, gun = group_unit[gi]
            if bi == meta["ggroups"][gi][0]:
                # group start: stream the S slab + issue the group's gathers
                sband = sld.tile([P, gun * 32], I32, name="sband",
                                 tag="sband",
                                 padded_shape=[P, max_group_unit * 32])
                nc.sync.dma_start(out=sband[:],
                                  in_=s_d[:, gu0 * 32:(gu0 + gun) * 32])
                sb8 = sband[:].bitcast(F8)
                for ci, c in enumerate(calls):
                    if c["group"] == gi:
                        issue_call(l, ci)
            if l == 0:
                # just-in-time chunk of the transposed input features
                lo = band[0] * P
                hi = min((band[-1] + 1) * P, shard_pad)
                nc.sync.dma_start(out=hT[0][:, lo:hi], in_=xT[:, lo:hi])

            abanks = [apool.tile([P, 512], F32, name="agg", tag="agg")
                      for _ in range((len(band) + 3) // 4)]
            bbanks = [bpool.tile([P, 512], F32, name="outp", tag="outp")
                      for _ in range((len(band) + 3) // 4)]
            for j, r in enumerate(band):
                aslice = abanks[j // 4][:, (j % 4) * P:(j % 4) * P + aw]
                units = per_range[r]
                for k, (ci, u, t) in enumerate(units):
                    g = gtiles[(l, ci)]
                    su = u - gu0
                    rhs = (g[:, t * 64:(t + 1) * 64].bitcast(F16)
                           if l == 0 else g[:, t, :aw])
                    nc.tensor.matmul(
                        out=aslice, lhsT=sb8[:, su * P:(su + 1) * P],
                        rhs=rhs, start=(k == 0),
                        stop=(k == len(units) - 1))
                # mean scaling: sa = agg * (1/deg), per-partition scalar
                sa = rpool.tile([P, aw], F16, name="sa", tag="sa",
                                padded_shape=[P, P])
                nc.vector.tensor_scalar(
                    out=sa[:], in0=aslice, scalar1=invd_sb[:, r:r + 1],
                    scalar2=None, op0=mybir.AluOpType.mult)
                if l == 0:
                    # transpose scaled agg, then W_neigh0^T @ aggT
                    paT = tpool.tile([P, 512], F16, name="paT", tag="pt")
                    nc.tensor.transpose(out=paT[:P, :P], in_=sa[:],
                                        identity=ident[:])
                    saT = rpool.tile([P, P], F16, name="saT", tag="saT")
                    nc.vector.tensor_copy(out=saT[:], in_=paT[:P, :P])
                    bslice = bbanks[j // 4][:, (j % 4) * P:(j % 4 + 1) * P]
                    nc.tensor.matmul(out=bslice, lhsT=Wn[0][:], rhs=saT[:],
                                     start=True, stop=False)
                    nc.tensor.matmul(out=bslice, lhsT=Ws[0][:],
                                     rhs=hT[0][:, r * P:(r + 1) * P],
                                     start=False, stop=True)
                    nc.scalar.activation(
                        out=hT[1][:, r * P:(r + 1) * P], in_=bslice,
                        func=mybir.ActivationFunctionType.Relu,
                        bias=w_sb["b0"][:])
                    emit_z(1, r, bi)
                elif l == 1:
                    # feature-major: B = sa^T + Ws1^T @ hT1
                    bslice = bbanks[j // 4][:, (j % 4) * P:(j % 4 + 1) * P]
                    nc.tensor.matmul(out=bslice, lhsT=sa[:], rhs=ident[:],
                                     start=True, stop=False)
                    nc.tensor.matmul(out=bslice, lhsT=Ws[1][:],
                                     rhs=hT[1][:, r * P:(r + 1) * P],
                                     start=False, stop=True)
                    nc.scalar.activation(
                        out=hT[2][:, r * P:(r + 1) * P], in_=bslice,
                        func=mybir.ActivationFunctionType.Relu,
                        bias=w_sb["b1"][:])
                    emit_z(2, r, bi)
                else:
                    # node-major final layer: B = sa + ones@b2 + hT2^T@Ws2
                    bslice = bbanks[j // 4][:, (j % 4) * P:(j % 4) * P + NC]
                    nc.tensor.matmul(out=bslice, lhsT=ident[:], rhs=sa[:],
                                     start=True, stop=False)
                    nc.tensor.matmul(out=bslice, lhsT=ones_sb[:],
                                     rhs=w_sb["b2"][:], start=False,
                                     stop=False)
                    nc.tensor.matmul(out=bslice,
                                     lhsT=hT[2][:, r * P:(r + 1) * P],
                                     rhs=Ws[2][:], start=False, stop=True)
                    rowt = rpool.tile([P, NC], F32, name="rowt", tag="rowt")
                    nc.scalar.activation(
                        out=rowt[:], in_=bslice,
                        func=mybir.ActivationFunctionType.Copy)
                    r0 = r * P
                    nrows = min(shard - r0, P)
                    if nrows > 0:
                        nc.sync.dma_start(out=out_d[r0:r0 + nrows, :],
                                          in_=rowt[:nrows, :])


# ----------------------------------------------------------------- runner
N_CORES = 8
N_NODES = 100000
N_EDGES = 600000
IN_FEATS = 128
N_HIDDEN = 128
N_CLASSES = 64

_TRACE_RESULT = {}


def kernel(x, src, dst, W_self0, W_neigh0, b0, W_self1, W_neigh1, b1,
           W_self2, W_neigh2, b2):
    import concourse.bacc as bacc
    from concourse import bass_utils

    x = np.asarray(x, np.float32)
    src = np.asarray(src, np.int64)
    dst = np.asarray(dst, np.int64)
    assert x.shape == (N_NODES, IN_FEATS)
    assert src.shape == (N_EDGES,) and dst.shape == (N_EDGES,)

    meta, per_core = prepare(x, src, dst, n_cores=N_CORES)

    wpack = {
        "W_self0": np.asarray(W_self0, np.float16),
        "W_neigh0": np.asarray(W_neigh0, np.float16),
        "W_self1": np.asarray(W_self1, np.float16),
        "W_neigh1": np.asarray(W_neigh1, np.float16),
        "W_self2": np.asarray(W_self2, np.float16),
        "W_neigh2": np.asarray(W_neigh2, np.float16),
        "b0": np.asarray(b0, np.float32).reshape(-1, 1),
        "b1": np.asarray(b1, np.float32).reshape(-1, 1),
        "b2": np.asarray(b2, np.float16).reshape(1, -1),
    }

    nc = bacc.Bacc("TRN2", target_bir_lowering=False, debug=False,
                   num_devices=N_CORES, num_swdge_queues=4)
    build_kernel(nc, meta, N_CLASSES)
    nc.compile()

    in_maps = []
    for c in range(N_CORES):
        pc = per_core[c]
        im = dict(xT=pc["xT"], idx_img=pc["idx_img"], s_img=pc["s_img"],
                  invd=pc["invd"], ones1=pc["ones1"], xg_img=pc["xg_img"])
        im.update(wpack)
        in_maps.append(im)

    trace = os.environ.get("SAGE_TRACE") == "1"
    res = bass_utils.run_bass_kernel_spmd(
        nc, in_maps, core_ids=list(range(N_CORES)), trace=trace)
    if trace:
        _TRACE_RESULT["exec_time_ns"] = res.exec_time_ns

    out = np.concatenate([res.results[c]["out"] for c in range(N_CORES)], 0)
    return np.ascontiguousarray(out[:N_NODES], np.float32)


# revision 54
# speedup vs baseline: 1.1780x; 1.0157x over previous
"""Trainium2 Bass kernel for a 3-layer distributed GraphSAGE
(100000 nodes, 600000 edges, feats 128 -> 128 -> 128 -> 64, mean aggregation).

Strategy: 8-way contiguous node partition.  Layer 0 gathers raw x rows
from host-staged window tables (no collective needed); layers 1-2
compute z = h@W_neigh per shard, replicate z with four window
AllGathers, gather the z rows for the in-edges with batched dma_gather
calls, and segment-sum them into PSUM via 0/1 selection-matrix matmuls.

v4 structure:
 - S selection matrices are 0/1 fp8 (half the stream of fp16); the 1/deg
   mean scaling is applied per dst range on the DVE from a resident
   table.  The slab is streamed per band as int32 so the DMA splitter
   emits 4x bigger descriptors.
 - Layers 0-1 accumulate the output PSUM feature-major (out = W^T @ hT),
   fusing bias+ReLU into one ACT op that writes hT[l+1] directly.
 - Layer 2 only computes/stores the first 64 z/out columns.
 - Uneven gather windows [30,30,30,8 ranges]: the last window's
   AllGather -- the one on the layer-boundary critical path -- is tiny.
 - xT loads are chunked per band; z-row stores go through the ACT HWDGE
   ring to keep the Sync ring free for S-band streams.
"""
import os
import sys

sys.path.insert(0, "/opt/trn_rl_repo")

import numpy as np


import concourse.bass as bass
import concourse.mybir as mybir
import concourse.tile as tile
from concourse.masks import make_identity

F32 = mybir.dt.float32
F16 = mybir.dt.float16
F8 = mybir.dt.float8e4
I16 = mybir.dt.int16
I32 = mybir.dt.int32
NP_F8 = mybir.dt.np(F8)

WINDOW_RANGES = [30, 30, 30, 8]


def _roundup(a, m):
    return (a + m - 1) // m * m


# ---------------------------------------------------------------- host prep
def prepare(x, src, dst, n_cores=8, band_ranges=8):
    n_nodes, in_feats = x.shape
    src = np.asarray(src, np.int64)
    dst = np.asarray(dst, np.int64)
    assert n_nodes % n_cores == 0
    shard = n_nodes // n_cores
    shard_pad = _roundup(shard, 128)
    n_ranges = shard_pad // 128
    assert sum(WINDOW_RANGES) == n_ranges
    q_ranges = []
    pos = 0
    for k in WINDOW_RANGES:
        q_ranges.append(list(range(pos, pos + k)))
        pos += k
    nw = len(q_ranges)
    q_rows = [len(qq) * 128 for qq in q_ranges]          # rows/core/window
    q_row0 = [qq[0] * 128 for qq in q_ranges]
    tbl_q = [n_cores * rw for rw in q_rows]
    assert all(t <= 32768 for t in tbl_q)

    deg = np.bincount(dst, minlength=n_nodes).astype(np.float32)
    inv_deg = (1.0 / np.maximum(deg, 1.0)).astype(np.float32)

    s_core = src // shard
    s_loc = src % shard
    w_ends = np.array([q_row0[w] + q_rows[w] for w in range(nw)])
    win_of = np.searchsorted(w_ends, s_loc, side="right")
    q_rows_a = np.asarray(q_rows)[win_of]
    q_row0_a = np.asarray(q_row0)[win_of]
    idx_in_win = s_core * q_rows_a + (s_loc - q_row0_a)

    core_of = dst // shard
    d_loc = dst - core_of * shard
    rng_of = d_loc // 128

    counts = np.zeros((n_cores, n_ranges, nw), np.int64)
    np.add.at(counts, (core_of, rng_of, win_of), 1)
    asz = counts.max(axis=0)                  # common section sizes, unrounded

    bands = [list(range(b, min(b + band_ranges, n_ranges)))
             for b in range(0, n_ranges, band_ranges)]
    # Gather groups: one band per group (call granularity == band).
    ggroups = [[g] for g in range(len(bands))]
    group_of_band = {}
    for gi, gg in enumerate(ggroups):
        for b in gg:
            group_of_band[b] = gi
    calls = []
    slot_cursor = 0
    n_units = 0
    for gi, gg in enumerate(ggroups):
        granges = [r for b in gg for r in bands[b]]
        for w in range(nw):
            sec = int(sum(asz[r, w] for r in granges))
            if sec == 0:
                continue
            nslots = _roundup(sec, 128)
            secs, off = [], 0
            for r in granges:
                if asz[r, w]:
                    secs.append((int(r), off, off + int(asz[r, w])))
                    off += int(asz[r, w])
            units = []       # (local subtile, range, unit id)
            for (r, lo, hi) in secs:
                for t in range(lo // 128, (hi + 127) // 128):
                    units.append((t, r, n_units))
                    n_units += 1
            calls.append(dict(w=w, slot0=slot_cursor, nslots=nslots,
                              secs=secs, units=units, group=gi))
            slot_cursor += nslots
    total_slots = slot_cursor

    # per-range schedule: (call idx, unit id, local subtile), in w order
    per_range = [[] for _ in range(n_ranges)]
    for ci, call in enumerate(calls):
        for (t, r, u) in call["units"]:
            per_range[r].append((ci, u, t))

    group_unit = []
    for gi in range(len(ggroups)):
        cs = [c for c in calls if c["group"] == gi]
        us = [u for c in cs for (_, _, u) in c["units"]]
        group_unit.append((min(us), max(us) - min(us) + 1))

    per_core = []
    for c in range(n_cores):
        m = core_of == c
        e_idx = idx_in_win[m]
        e_dl = d_loc[m]
        e_w = win_of[m]
        key = (e_dl // 128) * nw + e_w
        order = np.argsort(key, kind="stable")
        e_idx, e_dl, e_w = (a[order] for a in (e_idx, e_dl, e_w))

        e_src = src[m][order]           # global src node per edge

        idx16 = np.zeros(total_slots, np.int16)
        gsrc = np.zeros(total_slots, np.int64)
        S = np.zeros((n_units, 128, 128), NP_F8)
        cnt = counts[c]
        pos = 0
        gstart = {}
        for r in range(n_ranges):
            for w in range(nw):
                gstart[(r, w)] = pos
                pos += int(cnt[r, w])
        assert pos == m.sum()
        one8 = NP_F8(1.0)
        for ci, call in enumerate(calls):
            w = call["w"]
            s0 = call["slot0"]
            u_of = {(t, r): u for (t, r, u) in call["units"]}
            for (r, lo, hi) in call["secs"]:
                k = int(cnt[r, w])
                if k == 0:
                    continue
                e0 = gstart[(r, w)]
                sl = slice(e0, e0 + k)
                slots = np.arange(lo, lo + k)
                idx16[s0 + lo:s0 + lo + k] = e_idx[sl].astype(np.int16)
                gsrc[s0 + lo:s0 + lo + k] = e_src[sl]
                t0 = lo // 128
                uids = np.array([u_of[(t, r)] for t in
                                 range(t0, (hi + 127) // 128)])
                u_arr = uids[slots // 128 - t0]
                S[u_arr, slots % 128, (e_dl[sl] - r * 128)] = one8

        idx_img = np.tile(idx16.reshape(-1, 16).T, (8, 1))
        s_img = np.ascontiguousarray(
            S.transpose(1, 0, 2).reshape(128, n_units * 128)
        ).view(np.int32)
        # host-pregathered layer-0 rows, in slot order, g-tile layout:
        # partition p column-block s holds x[src(slot s*128+p)]
        x16g = x.astype(np.float16)[gsrc]           # [total_slots, 128]
        xg_img = np.ascontiguousarray(
            x16g.reshape(total_slots // 128, 128, 128)
            .transpose(1, 0, 2).reshape(128, total_slots * 128 // 128)
        ).view(np.int32)

        invd = np.ones((128, n_ranges), np.float32)
        base = c * shard
        for r in range(n_ranges):
            lo = r * 128
            hi = min(lo + 128, shard)
            if hi > lo:
                invd[: hi - lo, r] = inv_deg[base + lo: base + hi]

        xt = np.zeros((in_feats, shard_pad), np.float16)
        xt[:, :shard] = x[c * shard:(c + 1) * shard].T.astype(np.float16)
        per_core.append(dict(xT=xt, idx_img=idx_img, s_img=s_img, invd=invd,
                             xg_img=xg_img))

    ones1 = np.ones((1, 128), np.float16)
    for pc in per_core:
        pc["ones1"] = ones1

    meta = dict(n_cores=n_cores, shard=shard, shard_pad=shard_pad,
                n_ranges=n_ranges, q_ranges=q_ranges, q_rows=q_rows,
                q_row0=q_row0, tbl_q=tbl_q, nw=nw, bands=bands, calls=calls,
                per_range=per_range, group_unit=group_unit, n_units=n_units,
                ggroups=ggroups, group_of_band=group_of_band,
                total_slots=total_slots, in_feats=in_feats)
    return meta, per_core


# ------------------------------------------------------------- kernel build
def build_kernel(nc, meta, n_classes):
    P = 128
    shard, shard_pad = meta["shard"], meta["shard_pad"]
    fin0 = meta["in_feats"]

    xT = nc.dram_tensor("xT", [fin0, shard_pad], F16, kind="ExternalInput").ap()
    xg_d = nc.dram_tensor("xg_img", [P, meta["total_slots"] // 2], I32,
                          kind="ExternalInput").ap()
    idx_d = nc.dram_tensor("idx_img", [P, meta["total_slots"] // 16], I16,
                           kind="ExternalInput").ap()
    s_d = nc.dram_tensor("s_img", [P, meta["n_units"] * 32], I32,
                         kind="ExternalInput").ap()
    invd_d = nc.dram_tensor("invd", [P, meta["n_ranges"]], F32,
                            kind="ExternalInput").ap()
    ones_d = nc.dram_tensor("ones1", [1, P], F16, kind="ExternalInput").ap()
    w_specs = [("W_self0", [P, P], F16), ("W_neigh0", [P, P], F16),
               ("W_self1", [P, P], F16), ("W_neigh1", [P, P], F16),
               ("W_self2", [P, n_classes], F16),
               ("W_neigh2", [P, n_classes], F16),
               ("b0", [P, 1], F32), ("b1", [P, 1], F32),
               ("b2", [1, n_classes], F16)]
    w_d = {name: (nc.dram_tensor(name, shape, dt, kind="ExternalInput").ap(),
                  shape, dt)
           for name, shape, dt in w_specs}
    out_d = nc.dram_tensor("out", [shard, n_classes], F32,
                           kind="ExternalOutput").ap()

    with tile.TileContext(nc) as tc:
        import contextlib
        with contextlib.ExitStack() as ctx:
            _body(ctx, tc, meta, n_classes, xT, xg_d, idx_d, s_d, invd_d,
                  ones_d, w_d, out_d)
    return nc


def _body(ctx, tc, meta, n_classes, xT, xg_d, idx_d, s_d, invd_d, ones_d,
          w_d, out_d):
    P = 128
    NC = n_classes
    nc = tc.nc
    shard, shard_pad = meta["shard"], meta["shard_pad"]
    n_ranges = meta["n_ranges"]
    q_ranges, q_rows = meta["q_ranges"], meta["q_rows"]
    tbl_q, nw = meta["tbl_q"], meta["nw"]
    q_of_range = {}
    for q, qq in enumerate(q_ranges):
        for r in qq:
            q_of_range[r] = q
    q_last_range = [qq[-1] for qq in q_ranges]
    calls, per_range = meta["calls"], meta["per_range"]
    group_unit = meta["group_unit"]
    group_of_band = meta["group_of_band"]
    total_slots = meta["total_slots"]
    rg = [list(range(meta["n_cores"]))]
    max_call_sub = max(c["nslots"] for c in calls) // 128
    max_group_unit = max(n for _, n in group_unit)

    pers = ctx.enter_context(tc.tile_pool(name="pers", bufs=1))
    dram = ctx.enter_context(tc.tile_pool(name="dram", bufs=1, space="DRAM"))
    gpool = ctx.enter_context(tc.tile_pool(name="gp", bufs=28))
    sld = ctx.enter_context(tc.tile_pool(name="sld", bufs=2))
    hpool = ctx.enter_context(tc.tile_pool(name="hp", bufs=2))
    rpool = ctx.enter_context(tc.tile_pool(name="rp", bufs=4))
    apool = ctx.enter_context(tc.tile_pool(name="ap", bufs=2, space="PSUM"))
    bpool = ctx.enter_context(tc.tile_pool(name="bp", bufs=2, space="PSUM"))
    tpool = ctx.enter_context(tc.tile_pool(name="tp", bufs=2, space="PSUM"))
    zpool = ctx.enter_context(tc.tile_pool(name="zp", bufs=2, space="PSUM"))

    # warmup collective first: its input DMA must not queue behind the
    # bulk prelude loads, so the CC core is free before the first z AG.
    wu_in = dram.tile([P, 1], F32, name="wu_in")
    wu_out = dram.tile([P * meta["n_cores"], 1], F32, addr_space="Shared",
                       name="wu_out")
    wu_sb = pers.tile([P, 1], F32, name="wu_sb")
    nc.vector.memset(wu_sb[:], 0.0)
    nc.sync.dma_start(out=wu_in[:], in_=wu_sb[:])
    nc.gpsimd.collective_compute("AllGather", mybir.AluOpType.bypass,
                                 replica_groups=rg, ins=[wu_in[:]],
                                 outs=[wu_out[:]])

    idx_sb = pers.tile([P, total_slots // 16], I16, name="idx_sb")
    nc.sync.dma_start(out=idx_sb[:], in_=idx_d[:])
    invd_sb = pers.tile([P, n_ranges], F32, name="invd_sb")
    nc.sync.dma_start(out=invd_sb[:], in_=invd_d[:])
    ones_sb = pers.tile([1, P], F16, name="ones_sb")
    nc.sync.dma_start(out=ones_sb[:], in_=ones_d[:])
    ident = pers.tile([P, P], F16, name="ident")
    make_identity(nc, ident[:])
    w_sb = {}
    for name, (ap_, shape, dt) in w_d.items():
        t = pers.tile(shape, dt, name=name)
        nc.sync.dma_start(out=t[:], in_=ap_[:])
        w_sb[name] = t
    Ws = [w_sb["W_self0"], w_sb["W_self1"], w_sb["W_self2"]]
    Wn = [w_sb["W_neigh0"], w_sb["W_neigh1"], w_sb["W_neigh2"]]

    hT = [None] * 3
    hT[0] = hpool.tile([P, shard_pad], F16, name="hT0", tag="hT")
    for l in range(1, 3):
        hT[l] = hpool.tile([P, shard_pad], F16, name=f"hT{l}", tag="hT")

    # z tables (gather rows must stride 256B, so both are 128-wide fp16;
    # layer 2 only touches the first 64 columns).
    zw = [None, P, NC]                 # useful width per layer
    zbq = [None] + [[dram.tile([q_rows[q], P], F16, name=f"zb{l}_{q}")
                     for q in range(nw)] for l in range(1, 3)]
    zfq_t = [None] + [[dram.tile([tbl_q[q], P], F16, addr_space="Shared",
                                 name=f"zf{l}_{q}")
                       for q in range(nw)] for l in range(1, 3)]

    pending_ag = []     # deferred AllGather triggers: [z_layer, q, band]
    qn = [0]
    gtiles = {}         # (layer, call idx) -> gather tile
    call_of = {(c["group"], c["w"]): ci for ci, c in enumerate(calls)}

    def issue_call(gl, ci):
        """Issue the slot-data fetch for (layer gl, call ci) once."""
        if (gl, ci) in gtiles:
            return
        c = calls[ci]
        nsub_c = c["nslots"] // 128
        if gl == 0:
            # layer 0: stream host-pregathered rows (no SWDGE)
            g = gpool.tile([P, nsub_c * 64], I32, name="g", tag="g",
                           padded_shape=[P, max_call_sub * 64])
            nc.sync.dma_start(out=g[:],
                              in_=xg_d[:, c["slot0"] // 2:
                                       (c["slot0"] + c["nslots"]) // 2])
        else:
            g = gpool.tile([P, nsub_c, P], F16, name="g", tag="g",
                           padded_shape=[P, max_call_sub, P])
            nc.gpsimd.dma_gather(
                out_ap=g[:], in_ap=zfq_t[gl][c["w"]][:, :],
                idxs_ap=idx_sb[:, c["slot0"] // 16:
                               (c["slot0"] + c["nslots"]) // 16],
                num_idxs=c["nslots"], num_idxs_reg=c["nslots"],
                elem_size=P, single_packet=False,
                queue_num=qn[0] % nc.num_swdge_queues)
            qn[0] += 1
        gtiles[(gl, ci)] = g

    def ag_trigger(zl, q):
        nc.gpsimd.collective_compute(
            "AllGather", mybir.AluOpType.bypass, replica_groups=rg,
            ins=[zbq[zl][q][:]], outs=[zfq_t[zl][q][:]])

    def emit_z(l, r, bi):
        """Produce z_l rows for range r of h_l.  z1's window AllGathers
        fire immediately (the z1 chain is the l0-phase floor); z2's are
        deferred two bands so the trigger's store-completion wait never
        blocks the Pool queue behind pending gathers."""
        width = zw[l]
        pz = zpool.tile([P, 512], F32, name="pz", tag="pz")
        nc.tensor.matmul(out=pz[:, :width], lhsT=hT[l][:, r * P:(r + 1) * P],
                         rhs=Wn[l][:, :width], start=True, stop=True)
        zrow = rpool.tile([P, width], F16, name="zrow", tag="zrow",
                          padded_shape=[P, P])
        nc.scalar.activation(out=zrow[:], in_=pz[:, :width],
                             func=mybir.ActivationFunctionType.Copy)
        q = q_of_range[r]
        r2 = r - q_ranges[q][0]
        nc.scalar.dma_start(out=zbq[l][q][r2 * P:(r2 + 1) * P, :width],
                            in_=zrow[:])
        if r == q_last_range[q]:
            if l == 1 or q == nw - 1:
                ag_trigger(l, q)
            else:
                pending_ag.append([l, q, bi])

    for l in range(3):
        last = l == 2
        aw = NC if last else P          # aggregation width
        for bi, band in enumerate(meta["bands"]):
            # fire deferred AllGather triggers whose stores are long done
            for ent in list(pending_ag):
                if ent[0] <= l or ent[2] <= bi - 2:
                    ag_trigger(ent[0], ent[1])
                    pending_ag.remove(ent)
            gi = group_of_band[bi]
            gu0, gun = group_unit[gi]
            if bi == meta["ggroups"][gi][0]:
                # group start: stream the S slab + issue the group's gathers
                sband = sld.tile([P, gun * 32], I32, name="sband",
                                 tag="sband",
                                 padded_shape=[P, max_group_unit * 32])
                nc.sync.dma_start(out=sband[:],
                                  in_=s_d[:, gu0 * 32:(gu0 + gun) * 32])
                sb8 = sband[:].bitcast(F8)
                for ci, c in enumerate(calls):
                    if c["group"] == gi:
                        issue_call(l, ci)
            if l == 0:
                # just-in-time chunk of the transposed input features
                lo = band[0] * P
                hi = min((band[-1] + 1) * P, shard_pad)
                nc.sync.dma_start(out=hT[0][:, lo:hi], in_=xT[:, lo:hi])

            abanks = [apool.tile([P, 512], F32, name="agg", tag="agg")
                      for _ in range((len(band) + 3) // 4)]
            bbanks = [bpool.tile([P, 512], F32, name="outp", tag="outp")
                      for _ in range((len(band) + 3) // 4)]
            for j, r in enumerate(band):
                aslice = abanks[j // 4][:, (j % 4) * P:(j % 4) * P + aw]
                units = per_range[r]
                for k, (ci, u, t) in enumerate(units):
                    g = gtiles[(l, ci)]
                    su = u - gu0
                    rhs = (g[:, t * 64:(t + 1) * 64].bitcast(F16)
                           if l == 0 else g[:, t, :aw])
                    nc.tensor.matmul(
                        out=aslice, lhsT=sb8[:, su * P:(su + 1) * P],
                        rhs=rhs, start=(k == 0),
                        stop=(k == len(units) - 1))
                # mean scaling: sa = agg * (1/deg), per-partition scalar
                sa = rpool.tile([P, aw], F16, name="sa", tag="sa",
                                padded_shape=[P, P])
                nc.vector.tensor_scalar(
                    out=sa[:], in0=aslice, scalar1=invd_sb[:, r:r + 1],
                    scalar2=None, op0=mybir.AluOpType.mult)
                if l == 0:
                    # transpose scaled agg, then W_neigh0^T @ aggT
                    paT = tpool.tile([P, 512], F16, name="paT", tag="pt")
                    nc.tensor.transpose(out=paT[:P, :P], in_=sa[:],
                                        identity=ident[:])
                    saT = rpool.tile([P, P], F16, name="saT", tag="saT")
                    nc.vector.tensor_copy(out=saT[:], in_=paT[:P, :P])
                    bslice = bbanks[j // 4][:, (j % 4) * P:(j % 4 + 1) * P]
                    nc.tensor.matmul(out=bslice, lhsT=Wn[0][:], rhs=saT[:],
                                     start=True, stop=False)
                    nc.tensor.matmul(out=bslice, lhsT=Ws[0][:],
                                     rhs=hT[0][:, r * P:(r + 1) * P],
                                     start=False, stop=True)
                    nc.scalar.activation(
                        out=hT[1][:, r * P:(r + 1) * P], in_=bslice,
                        func=mybir.ActivationFunctionType.Relu,
                        bias=w_sb["b0"][:])
                    emit_z(1, r, bi)
                elif l == 1:
                    # feature-major: B = sa^T + Ws1^T @ hT1
                    bslice = bbanks[j // 4][:, (j % 4) * P:(j % 4 + 1) * P]
                    nc.tensor.matmul(out=bslice, lhsT=sa[:], rhs=ident[:],
                                     start=True, stop=False)
                    nc.tensor.matmul(out=bslice, lhsT=Ws[1][:],
                                     rhs=hT[1][:, r * P:(r + 1) * P],
                                     start=False, stop=True)
                    nc.scalar.activation(
                        out=hT[2][:, r * P:(r + 1) * P], in_=bslice,
                        func=mybir.ActivationFunctionType.Relu,
                        bias=w_sb["b1"][:])
                    emit_z(2, r, bi)
                else:
                    # node-major final layer: B = sa + ones@b2 + hT2^T@Ws2
                    bslice = bbanks[j // 4][:, (j % 4) * P:(j % 4) * P + NC]
                    nc.tensor.matmul(out=bslice, lhsT=ident[:], rhs=sa[:],
                                     start=True, stop=False)
                    nc.tensor.matmul(out=bslice, lhsT=ones_sb[:],
                                     rhs=w_sb["b2"][:], start=False,
                                     stop=False)
                    nc.tensor.matmul(out=bslice,
                                     lhsT=hT[2][:, r * P:(r + 1) * P],
                                     rhs=Ws[2][:], start=False, stop=True)
                    rowt = rpool.tile([P, NC], F32, name="rowt", tag="rowt")
                    nc.scalar.activation(
                        out=rowt[:], in_=bslice,
                        func=mybir.ActivationFunctionType.Copy)
                    r0 = r * P
                    nrows = min(shard - r0, P)
                    if nrows > 0:
                        nc.sync.dma_start(out=out_d[r0:r0 + nrows, :],
                                          in_=rowt[:nrows, :])


# ----------------------------------------------------------------- runner
N_CORES = 8
N_NODES = 100000
N_EDGES = 600000
IN_FEATS = 128
N_HIDDEN = 128
N_CLASSES = 64

_TRACE_RESULT = {}


def kernel(x, src, dst, W_self0, W_neigh0, b0, W_self1, W_neigh1, b1,
           W_self2, W_neigh2, b2):
    import concourse.bacc as bacc
    from concourse import bass_utils

    x = np.asarray(x, np.float32)
    src = np.asarray(src, np.int64)
    dst = np.asarray(dst, np.int64)
    assert x.shape == (N_NODES, IN_FEATS)
    assert src.shape == (N_EDGES,) and dst.shape == (N_EDGES,)

    meta, per_core = prepare(x, src, dst, n_cores=N_CORES)

    wpack = {
        "W_self0": np.asarray(W_self0, np.float16),
        "W_neigh0": np.asarray(W_neigh0, np.float16),
        "W_self1": np.asarray(W_self1, np.float16),
        "W_neigh1": np.asarray(W_neigh1, np.float16),
        "W_self2": np.asarray(W_self2, np.float16),
        "W_neigh2": np.asarray(W_neigh2, np.float16),
        "b0": np.asarray(b0, np.float32).reshape(-1, 1),
        "b1": np.asarray(b1, np.float32).reshape(-1, 1),
        "b2": np.asarray(b2, np.float16).reshape(1, -1),
    }

    nc = bacc.Bacc("TRN2", target_bir_lowering=False, debug=False,
                   num_devices=N_CORES, num_swdge_queues=4)
    build_kernel(nc, meta, N_CLASSES)
    nc.compile()

    in_maps = []
    for c in range(N_CORES):
        pc = per_core[c]
        im = dict(xT=pc["xT"], idx_img=pc["idx_img"], s_img=pc["s_img"],
                  invd=pc["invd"], ones1=pc["ones1"], xg_img=pc["xg_img"])
        im.update(wpack)
        in_maps.append(im)

    trace = os.environ.get("SAGE_TRACE") == "1"
    res = bass_utils.run_bass_kernel_spmd(
        nc, in_maps, core_ids=list(range(N_CORES)), trace=trace)
    if trace:
        _TRACE_RESULT["exec_time_ns"] = res.exec_time_ns

    out = np.concatenate([res.results[c]["out"] for c in range(N_CORES)], 0)
    return np.ascontiguousarray(out[:N_NODES], np.float32)
